# revision 35
# baseline (speedup 1.0000x reference)
"""Trainium2 Bass kernel for nn_Pixelwise_77919296684103.

Depth decode via structured two-harmonic model:
  BVals standardization + nearest-neighbor over a 10000-entry code table
  collapses to argmin over theta of S(t) = Uw cos2t + Vw sin2t + P cost + Q sint
  (per pixel, with the per-pixel std absorbed into Uw,Vw via w=sqrt(vv)),
  solved with a 32-point coarse-grid argmin (bf16 PE matmuls, batched
  find_index8 over 8-slab groups) + 1 full Newton step + 1 modified step
  (reused curvature). cos/sin via the Scalar-engine Sin activation
  (range-reduced); the Mod/Dem harmonic extraction runs on-device with an
  iota+Sin generated basis. Engine split: DVE/Pool coherent chains, ACT for
  trig/sqrt (activation tables prefetched via dummy ops), PE for grid
  scoring/broadcast/transpose.

Sharding: data-parallel over pixels. 19200 pixels -> 8 cores x 2400
(padded to 2432 = 128x19 tiles). Mod/Demod tables replicated per core.
"""
import numpy as np
import sys

for _p in ("/opt/trn_rl_repo",):
    if _p not in sys.path:
        sys.path.insert(0, _p)

from concourse import bass, mybir
import concourse.tile as tile_mod
import concourse.bass2jax as _b2j
from concourse.vector_clock import ScopedClock
from concourse.masks import make_identity
from concourse.bass_utils import run_bass_kernel_spmd

# ---------------------------------------------------------------------------
# Patches: this walrus build allows only ONE semaphore wait per instruction.
# 1) TileContext exit Drain: split its sem waits across NOPs.
# 2) Global BIR pass: hoist extra waits onto NoOps before the owner.
# ---------------------------------------------------------------------------
if not getattr(tile_mod, "_onewait_patched", False):
    tile_mod._onewait_patched = True

    def _patched_drain_and_barrier(self, tick_clock, wait_clock):
        nc = self.nc
        probe = nc.sync.nop(nofuse=True)
        wait_clock.add_sem_waits(probe.ins, ScopedClock({None: tick_clock.global_clock}))
        si = probe.ins.sync_info
        waits = list(si.on_wait) if si is not None else []
        if len(waits) > 1:
            si.on_wait = waits[:1]
            for w in waits[1:]:
                nop = nc.sync.nop(nofuse=True)
                nop.ins.sync_info = mybir.SyncInfo(on_wait=[w], on_update=[])
        nc.sync.drain()
        nc.all_engine_barrier()
        assert self.sems is not None
        popped = nc._tile_sem_poison_stack.pop()
        assert popped is self._sem_poison
        nc.clear_and_free_semaphores(list(self.sems.allocated().values()))
        nc.all_engine_barrier()

    tile_mod.TileContext._drain_and_barrier = _patched_drain_and_barrier

    import json as _json

    _orig_decompress = _b2j._decompress_ant_bir

    def _fix_bir_bytes(raw: bytes) -> bytes:
        bir = _json.loads(raw)
        changed = False
        for fn in bir.get("functions", []):
            for bb in fn.get("blocks", []):
                newlist = []
                for ins in bb.get("instructions", []):
                    si = ins.get("sync_info")
                    waits = (si or {}).get("on_wait") or []
                    if len(waits) > 1:
                        changed = True
                        for j, wx in enumerate(waits[:-1]):
                            newlist.append({
                                "debug": ins.get("debug"),
                                "engine": ins["engine"],
                                "ins": [],
                                "name": f"{ins['name']}w{j}",
                                "opcode": "NoOp",
                                "outs": [],
                                "sync_info": {"on_update": [], "on_wait": [wx]},
                            })
                        si["on_wait"] = waits[-1:]
                    newlist.append(ins)
                bb["instructions"] = newlist
        if not changed:
            return raw
        return _json.dumps(bir).encode()

    def _decompress_and_fix(data):
        return _fix_bir_bytes(_orig_decompress(data))

    _b2j._decompress_ant_bir = _decompress_and_fix

# ---------------------------------------------------------------------------
# Constants
# ---------------------------------------------------------------------------
f32 = mybir.dt.float32
i32 = mybir.dt.int32
u32 = mybir.dt.uint32
bf16 = mybir.dt.bfloat16
AX = mybir.AxisListType
OP = mybir.AluOpType
AF = mybir.ActivationFunctionType

nf32 = np.float32
N = 10000
K = 3
G = 32
NCORES = 8
PIX = 19200
PPC = 2400              # pixels per core
NT = 19                 # tiles of 128 per core (2432 padded)
C_LIGHT = 299792458.0 * 1000.0
TAU_MIN = 2.0 * 10000.0 / C_LIGHT
DT = float(nf32(TAU_MIN / N))
PA = float(nf32(1e6))
INV_N = float(nf32(1.0 / N))
CHAT2 = 2.0 * (N - 1) / N
CHAT = float(np.sqrt(CHAT2))
SQ2 = float(np.sqrt(2.0))
HUp = float(nf32(0.5 * CHAT2 / SQ2))        # U scale (w-absorbed)
HV2p = float(nf32(2.0 * (-0.5) * CHAT2 / SQ2))   # 2*V scale for ttr
R28 = float(nf32(2.0 ** 28))
NHP2 = float(nf32(2.0 * CHAT * R28))        # -(HP*2^28), HP = -2*CHAT
NHQ2 = float(nf32(-2.0 * CHAT * R28))       # -(HQ*2^28), HQ = +2*CHAT
SPACING = float(nf32(2.0 * np.pi / G))
NFC = float(nf32(N / (2.0 * np.pi)))
CADT = float(nf32(INV_N) * nf32(DT))
C2ADT = float(nf32(2.0) * nf32(INV_N) * nf32(DT))
NC2ADT = -C2ADT
PADT = float(nf32(PA) * nf32(DT))
C2, S3 = -0.5, float(nf32(-1.0 / 6.0))
SC_UP = float(nf32(2.0 ** 56))              # exact pow2 prescale for sqrt
TWO_PI_S = 6.283184
TWO_PI_T = float(nf32(2.0 * np.pi))                         # slightly < 2*pi: keeps |x| < pi
D2_CLAMP = 1e-12


# ---------------------------------------------------------------------------
# Device program
# ---------------------------------------------------------------------------
def _build():
    nc = bass.Bass()
    GIN = nc.dram_tensor("GIN", [128, NT], f32, kind="ExternalInput")
    COMB = nc.dram_tensor("COMB", [125, 480], f32, kind="ExternalInput")
    O125 = nc.dram_tensor("O125", [125, 1], f32, kind="ExternalInput")
    O1 = nc.dram_tensor("O1", [1, 128], f32, kind="ExternalInput")
    GRIDC = nc.dram_tensor("GRIDC", [2, G], bf16, kind="ExternalInput")
    C2G = nc.dram_tensor("C2G", [1, G], f32, kind="ExternalInput")
    S2G = nc.dram_tensor("S2G", [1, G], f32, kind="ExternalInput")
    OUT = nc.dram_tensor("OUT", [128, NT], f32, kind="ExternalOutput")

    with tile_mod.TileContext(nc) as tc:
        with tc.tile_pool(name="sb", bufs=1) as sb, \
             tc.tile_pool(name="ps", bufs=1, space="PSUM") as ps, \
             tc.tile_pool(name="ps2", bufs=3, space="PSUM") as ps2:
            P19 = [128, NT]
            vt = nc.vector.tensor_tensor
            vs = nc.vector.tensor_scalar
            vstt = nc.vector.scalar_tensor_tensor
            pt_ = nc.gpsimd.tensor_tensor
            ps_ = nc.gpsimd.tensor_scalar

            # ---- input DMAs: gin first on gpsimd; Mod/Dem halves on their
            # own queues so neither waits behind the other ----
            gin = sb.tile(P19, dtype=f32)
            nc.gpsimd.dma_start(out=gin[:], in_=GIN[:])
            comb = sb.tile([125, 480], dtype=f32)
            nc.sync.dma_start(out=comb[:, 0:240], in_=COMB[:, 0:240])
            nc.scalar.dma_start(out=comb[:, 240:480], in_=COMB[:, 240:480])
            o125 = sb.tile([125, 1], dtype=f32)
            nc.gpsimd.dma_start(out=o125[:], in_=O125[:])
            o1 = sb.tile([1, 128], dtype=f32)
            nc.gpsimd.dma_start(out=o1[:], in_=O1[:])
            # grid rows: [W2n; cos; sin]; W2n DMA'd in later (stage D)
            gridt = sb.tile([3, G], dtype=bf16)
            nc.sync.dma_start(out=gridt[1:3, :], in_=GRIDC[:])
            c2gt = sb.tile([1, G], dtype=f32)
            nc.scalar.dma_start(out=c2gt[:], in_=C2G[:])
            s2gt = sb.tile([1, G], dtype=f32)
            nc.scalar.dma_start(out=s2gt[:], in_=S2G[:])

            # ---- on-device cos/sin basis [125,80] (saves 240KB of DMA) ----
            iob = sb.tile([125, 80], dtype=i32)
            nc.gpsimd.iota(iob[:], pattern=[[1, 80]], base=0, channel_multiplier=80)
            iof = sb.tile([125, 80], dtype=f32)
            nc.vector.tensor_copy(iof[:], iob[:])
            tb_ = sb.tile([125, 80], dtype=f32)
            vs(tb_[:], iof[:], INV_N, None, OP.mult)
            msb = sb.tile([125, 80], dtype=f32)
            vs(msb[:], tb_[:], 0.5, None, OP.is_ge)
            wsb = sb.tile([125, 80], dtype=f32)
            vt(wsb[:], tb_[:], msb[:], OP.subtract)
            SB80 = sb.tile([125, 80], dtype=f32)
            nc.scalar.activation(SB80[:], wsb[:], AF.Sin, scale=TWO_PI_S)
            mcb = sb.tile([125, 80], dtype=f32)
            vs(mcb[:], tb_[:], 0.25, None, OP.is_ge)
            wcb = sb.tile([125, 80], dtype=f32)
            vstt(wcb[:], tb_[:], 0.25, mcb[:], OP.add, OP.subtract)
            CB80 = sb.tile([125, 80], dtype=f32)
            nc.scalar.activation(CB80[:], wcb[:], AF.Sin, scale=TWO_PI_S)

            # within-group slab index q = col%8, as f32 (for tg correction)
            qoi = sb.tile([128, NT], dtype=i32)
            nc.gpsimd.iota(qoi[:, 0:8], pattern=[[1, 8]], base=0, channel_multiplier=0)
            nc.gpsimd.iota(qoi[:, 8:16], pattern=[[1, 8]], base=0, channel_multiplier=0)
            nc.gpsimd.iota(qoi[:, 16:19], pattern=[[1, 3]], base=0, channel_multiplier=0)
            qof = sb.tile([128, NT], dtype=f32)
            nc.vector.tensor_copy(qof[:], qoi[:])

            # ---- FRONT: t = gin/N; cos/sin(2*pi*t) via Sin activation ----
            tpix = sb.tile(P19, dtype=f32)
            vs(tpix[:], gin[:], INV_N, None, OP.mult)
            msk = sb.tile(P19, dtype=f32)
            vs(msk[:], tpix[:], 0.5, None, OP.is_ge)
            wsin = sb.tile(P19, dtype=f32)
            vt(wsin[:], tpix[:], msk[:], OP.subtract)
            si = sb.tile(P19, dtype=f32)
            nc.scalar.activation(si[:], wsin[:], AF.Sin, scale=TWO_PI_S)
            mskc = sb.tile(P19, dtype=f32)
            vs(mskc[:], tpix[:], 0.25, None, OP.is_ge)
            wcos = sb.tile(P19, dtype=f32)
            vstt(wcos[:], tpix[:], 0.25, mskc[:], OP.add, OP.subtract)
            ci = sb.tile(P19, dtype=f32)
            nc.scalar.activation(ci[:], wcos[:], AF.Sin, scale=TWO_PI_S)
            # dummy: prefetch sqrt act-table while DVE/Pool run stage A
            junk11 = sb.tile([1, 1], dtype=f32)
            nc.scalar.activation(junk11[:], si[0:1, 0:1], AF.Sqrt, scale=0.0)

            # ---- stage A: harmonics 0/1 of Mod/Dem columns ----
            # RH cols: Mc1(0:3) Dc1(3:6) Msn1(6:9) Dsn1(9:12) Ms0(12:15) Ds0(15:18)
            RH = sb.tile([125, 18], dtype=f32)
            md4 = comb[:].rearrange("p (t c k) -> p t c k", t=2, k=3)
            cb4 = CB80[:].rearrange("p (a c b) -> p a c b", a=1, b=1).broadcast_to([125, 2, 80, 3])
            sb4 = SB80[:].rearrange("p (a c b) -> p a c b", a=1, b=1).broadcast_to([125, 2, 80, 3])
            pAC = sb.tile([125, 480], dtype=f32)
            vt(pAC[:].rearrange("p (t c k) -> p t c k", t=2, k=3), cb4, md4, OP.mult)
            pBD = sb.tile([125, 480], dtype=f32)
            pt_(pBD[:].rearrange("p (t c k) -> p t c k", t=2, k=3), sb4, md4, OP.mult)

            def red6(out_sl, in_t):
                nc.vector.tensor_reduce(
                    out=out_sl, in_=in_t.rearrange("p (t c k) -> p t k c", t=2, k=3),
                    axis=AX.X, op=OP.add)

            red6(RH[:, 0:6], pAC[:])                     # Mc1 | Dc1
            red6(RH[:, 6:12], pBD[:])                    # Msn1 | Dsn1
            red6(RH[:, 12:18], comb[:])                  # Ms0 | Ds0

            ps18 = ps.tile([1, 18], dtype=f32)
            nc.tensor.matmul(ps18[:], o125[:], RH[:], start=True, stop=True)
            H = sb.tile([1, 18], dtype=f32)
            nc.vector.tensor_copy(H[:], ps18[:])
            psb = ps.tile([128, 18], dtype=f32)
            nc.tensor.matmul(psb[:], o1[:], H[:], start=True, stop=True)
            HB = sb.tile([128, 18], dtype=f32)
            nc.vector.tensor_copy(HB[:], psb[:])
            Mc1, Dc1, Msn1 = HB[:, 0:3], HB[:, 3:6], HB[:, 6:9]
            Dsn1, Ms0, Ds0 = HB[:, 9:12], HB[:, 12:15], HB[:, 15:18]

            # ---- stage B (wide, after immediate broadcast) ----
            abkW = sb.tile([128, 3], dtype=f32)
            CkW = sb.tile([128, 3], dtype=f32)
            SkW = sb.tile([128, 3], dtype=f32)
            t2 = sb.tile([128, 3], dtype=f32)
            t3 = sb.tile([128, 3], dtype=f32)
            vt(t2[:], Mc1, Dc1, OP.mult)
            vt(t3[:], Msn1, Dsn1, OP.mult)
            vt(t2[:], t2[:], t3[:], OP.add)
            vs(CkW[:], t2[:], C2ADT, None, OP.mult)                  # Ck
            t4 = sb.tile([128, 3], dtype=f32)
            t5 = sb.tile([128, 3], dtype=f32)
            pt_(t4[:], Msn1, Dc1, OP.mult)
            pt_(t5[:], Mc1, Dsn1, OP.mult)
            pt_(t4[:], t4[:], t5[:], OP.subtract)
            ps_(SkW[:], t4[:], NC2ADT, None, OP.mult)                # Sk
            t6 = sb.tile([128, 3], dtype=f32)
            pt_(t6[:], Ms0, Ds0, OP.mult)
            xx = sb.tile([128, 3], dtype=f32)
            ps_(xx[:], Ds0, PADT, None, OP.mult)
            ps_(t6[:], t6[:], CADT, None, OP.mult)
            pt_(abkW[:], t6[:], xx[:], OP.add)                       # abk
            q1w = sb.tile([128, 3], dtype=f32)
            q2w = sb.tile([128, 3], dtype=f32)
            pt_(q1w[:], CkW, CkW, OP.mult)
            pt_(q2w[:], SkW, SkW, OP.mult)
            pt_(q1w[:], q1w[:], q2w[:], OP.add)
            rrw = sb.tile([128, 3], dtype=f32)
            nc.scalar.activation(rrw[:], q1w[:], AF.Sqrt, scale=SC_UP)
            riw = sb.tile([128, 3], dtype=f32)
            nc.vector.reciprocal(riw[:], rrw[:])         # = 2^-28 / r
            cwW = sb.tile([128, 3], dtype=f32)
            cwt = sb.tile([128, 3], dtype=f32)
            ps_(cwt[:], CkW, R28, None, OP.mult)
            pt_(cwW[:], cwt[:], riw[:], OP.mult)                     # cw
            swW = sb.tile([128, 3], dtype=f32)
            swt = sb.tile([128, 3], dtype=f32)
            ps_(swt[:], SkW, -R28, None, OP.mult)
            pt_(swW[:], swt[:], riw[:], OP.mult)                     # sw
            p1 = sb.tile([128, 3], dtype=f32)
            p2 = sb.tile([128, 3], dtype=f32)
            pt_(p1[:], cwW[:], cwW[:], OP.mult)
            pt_(p2[:], swW[:], swW[:], OP.mult)
            p3 = sb.tile([128, 3], dtype=f32)
            pt_(p3[:], p1[:], p2[:], OP.subtract)
            suW = sb.tile([128, 1], dtype=f32)
            nc.vector.tensor_reduce(out=suW[:], in_=p3[:], axis=AX.X, op=OP.add)
            UW = sb.tile([128, 1], dtype=f32)
            ps_(UW[:], suW[:], HUp, None, OP.mult)                   # U
            p4 = sb.tile([128, 3], dtype=f32)
            pt_(p4[:], cwW[:], swW[:], OP.mult)
            svW = sb.tile([128, 1], dtype=f32)
            nc.vector.tensor_reduce(out=svW[:], in_=p4[:], axis=AX.X, op=OP.add)
            VW = sb.tile([128, 1], dtype=f32)
            ps_(VW[:], svW[:], HV2p, None, OP.mult)                  # V

            # ---- stage D: grid row 0 = -(U*c2g + V*s2g) ----
            w2a = sb.tile([1, G], dtype=f32)
            nc.scalar.mul(w2a[:], c2gt[:], UW[0:1, :])
            w2b = sb.tile([1, G], dtype=f32)
            nc.scalar.mul(w2b[:], s2gt[:], VW[0:1, :])
            vt(w2a[:], w2a[:], w2b[:], OP.add)
            w2n = sb.tile([1, G], dtype=bf16)
            vs(w2n[:], w2a[:], -1.0, None, OP.mult)
            nc.sync.dma_start(out=gridt[0:1, :], in_=w2n[:])

            # ---- stage E: pixel path (coherent DVE/Pool chains) ----
            VB = [sb.tile(P19, dtype=f32, name=f"vb{_k}") for _k in range(K)]
            V = [sb.tile(P19, dtype=f32, name=f"vk{_k}") for _k in range(K)]
            for k in range(K):
                vstt(VB[k][:], si[:], SkW[:, k:k + 1],
                     abkW[:, k:k + 1].broadcast_to(P19), OP.mult, OP.add)
                vstt(V[k][:], ci[:], CkW[:, k:k + 1], VB[k][:], OP.mult, OP.add)
            S3s = sb.tile(P19, dtype=f32)
            pt_(S3s[:], V[0][:], V[1][:], OP.add)
            pt_(S3s[:], S3s[:], V[2][:], OP.add)
            E = [sb.tile(P19, dtype=f32, name=f"ek{_k}") for _k in range(K)]
            for k in range(K):
                vstt(E[k][:], V[k][:], 3.0, S3s[:], OP.mult, OP.subtract)
            z0 = sb.tile(P19, dtype=f32)
            z1 = sb.tile(P19, dtype=f32)
            z2 = sb.tile(P19, dtype=f32)
            vt(z0[:], E[0][:], E[0][:], OP.mult)
            pt_(z1[:], E[1][:], E[1][:], OP.mult)
            vt(z2[:], E[2][:], E[2][:], OP.mult)
            vv = sb.tile(P19, dtype=f32)
            pt_(vv[:], z0[:], z1[:], OP.add)
            pt_(vv[:], vv[:], z2[:], OP.add)
            # PQW cols: wq(0:19) -P2(19:38) -Q2(38:57)
            PQW = sb.tile([128, 3 * NT], dtype=f32)
            nc.scalar.activation(PQW[:, 0:NT], vv[:], AF.Sqrt, scale=SC_UP)
            wq = PQW[:, 0:NT]
            a0 = sb.tile(P19, dtype=f32)
            vt(a0[:], E[0][:], cwW[:, 0:1].broadcast_to(P19), OP.mult)
            Aac = sb.tile(P19, dtype=f32)
            vstt(Aac[:], E[1][:], cwW[:, 1:2], a0[:], OP.mult, OP.add)
            vstt(Aac[:], E[2][:], cwW[:, 2:3], Aac[:], OP.mult, OP.add)
            vs(PQW[:, NT:2 * NT], Aac[:], NHP2, None, OP.mult)       # -P2
            b0 = sb.tile(P19, dtype=f32)
            b1 = sb.tile(P19, dtype=f32)
            b2 = sb.tile(P19, dtype=f32)
            Bac = sb.tile(P19, dtype=f32)
            pt_(b0[:], E[0][:], swW[:, 0:1].broadcast_to(P19), OP.mult)
            pt_(b1[:], E[1][:], swW[:, 1:2].broadcast_to(P19), OP.mult)
            pt_(Bac[:], b0[:], b1[:], OP.add)
            pt_(b2[:], E[2][:], swW[:, 2:3].broadcast_to(P19), OP.mult)
            pt_(Bac[:], Bac[:], b2[:], OP.add)
            ps_(PQW[:, 2 * NT:3 * NT], Bac[:], NHQ2, None, OP.mult)  # -Q2
            nP2 = PQW[:, NT:2 * NT]
            nQ2 = PQW[:, 2 * NT:3 * NT]

            # ---- transpose -> slab lhsT rows [wq; -P2; -Q2] (bf16) ----
            ident = sb.tile([128, 128], dtype=f32)
            make_identity(nc, ident[:])
            ptp = ps.tile([3 * NT, 128], dtype=f32)
            nc.tensor.transpose(ptp[:], PQW[:], ident[:])
            Pt = sb.tile([3 * NT, 128], dtype=bf16)
            nc.vector.tensor_copy(Pt[:], ptp[:])
            NA = 10
            L3a = sb.tile([3, NA * 128], dtype=bf16)
            L3b = sb.tile([3, (NT - NA) * 128], dtype=bf16)
            nc.gpsimd.dma_start(out=L3a[0:1, :], in_=Pt[0:NA, :])
            nc.scalar.dma_start(out=L3a[1:2, :], in_=Pt[NT:NT + NA, :])
            nc.sync.dma_start(out=L3a[2:3, :], in_=Pt[2 * NT:2 * NT + NA, :])
            nc.gpsimd.dma_start(out=L3b[0:1, :], in_=Pt[NA:NT, :])
            nc.scalar.dma_start(out=L3b[1:2, :], in_=Pt[NT + NA:2 * NT, :])
            nc.sync.dma_start(out=L3b[2:3, :], in_=Pt[2 * NT + NA:3 * NT, :])

            # Newton coefficient tensors via [128,1] consts + broadcast-tt
            m2Uc = sb.tile([128, 1], dtype=f32)
            vs(m2Uc[:], UW[:], -2.0, None, OP.mult)
            m4Uc = sb.tile([128, 1], dtype=f32)
            vs(m4Uc[:], UW[:], -4.0, None, OP.mult)
            p2Vc = sb.tile([128, 1], dtype=f32)
            vs(p2Vc[:], VW[:], 2.0, None, OP.mult)
            m4Vc = sb.tile([128, 1], dtype=f32)
            vs(m4Vc[:], VW[:], -4.0, None, OP.mult)
            m2Uw = sb.tile(P19, dtype=f32)
            vt(m2Uw[:], wq, m2Uc[:].broadcast_to(P19), OP.mult)
            m4Vw = sb.tile(P19, dtype=f32)
            vt(m4Vw[:], wq, m4Vc[:].broadcast_to(P19), OP.mult)
            m4Uw = sb.tile(P19, dtype=f32)
            pt_(m4Uw[:], wq, m4Uc[:].broadcast_to(P19), OP.mult)
            p2Vw = sb.tile(P19, dtype=f32)
            pt_(p2Vw[:], wq, p2Vc[:].broadcast_to(P19), OP.mult)


            # ---- coarse argmin: 8-slab groups; one windowed reduce for the
            # 8 maxes, ONE find_index8 recovers all 8 argmax positions (global
            # in the 512-wide group; the 64*q offset is folded into tg). ----
            mi_all = sb.tile([128, 24], dtype=u32)
            groups = [(0, 8), (8, 8), (16, 3)]
            for gi, (g0, gn) in enumerate(groups):
                npsg = ps2.tile([128, gn * G], dtype=f32)
                for q in range(gn):
                    j = g0 + q
                    if j < NA:
                        lhs = L3a[:, 128 * j:128 * (j + 1)]
                    else:
                        lhs = L3b[:, 128 * (j - NA):128 * (j - NA + 1)]
                    nc.tensor.matmul(npsg[:, G * q:G * (q + 1)], lhs, gridt[:],
                                     start=True, stop=True)
                mxg = sb.tile([128, 8], dtype=f32, name=f"mxg{g0}")
                if gn < 8:
                    nc.vector.memset(mxg[:, gn:8], 0.0)
                nc.vector.tensor_reduce(
                    out=mxg[:, 0:gn], in_=npsg[:].rearrange("p (s g) -> p s g", g=G),
                    axis=AX.X, op=OP.max)
                nc.vector.max_index(mi_all[:, 8 * gi:8 * gi + 8], mxg[:], npsg[:])
            # dummy: prefetch trig act-table during the slab phase
            junk12 = sb.tile([1, 1], dtype=f32)
            nc.scalar.activation(junk12[:], PQW[0:1, 0:1], AF.Sin, scale=0.0)
            GF = sb.tile(P19, dtype=f32)
            nc.vector.tensor_copy(GF[:], mi_all[:].bitcast(i32)[:, 0:NT])

            # ---- cos/sin at grid point (DVE + ACT) ----
            tg = sb.tile(P19, dtype=f32)
            vstt(tg[:], GF[:], 1.0 / G, qof[:], OP.mult, OP.subtract)
            ms2 = sb.tile(P19, dtype=f32)
            vs(ms2[:], tg[:], 0.5, None, OP.is_ge)
            ws2 = sb.tile(P19, dtype=f32)
            vt(ws2[:], tg[:], ms2[:], OP.subtract)
            sgg = sb.tile(P19, dtype=f32)
            nc.scalar.activation(sgg[:], ws2[:], AF.Sin, scale=TWO_PI_S)
            mc2 = sb.tile(P19, dtype=f32)
            vs(mc2[:], tg[:], 0.25, None, OP.is_ge)
            wc2 = sb.tile(P19, dtype=f32)
            vstt(wc2[:], tg[:], 0.25, mc2[:], OP.add, OP.subtract)
            cgg = sb.tile(P19, dtype=f32)
            nc.scalar.activation(cgg[:], wc2[:], AF.Sin, scale=TWO_PI_S)
            thg = sb.tile(P19, dtype=f32)
            ps_(thg[:], tg[:], TWO_PI_T, None, OP.mult)

            # ---- Newton: iter0 delta=0; iter1 full rotation; iter2 linear
            #      rotation + reused reciprocal. D/P split per subtree. ----
            x2 = sb.tile(P19, dtype=f32)
            cd = sb.tile(P19, dtype=f32)
            ct = sb.tile(P19, dtype=f32)
            st = sb.tile(P19, dtype=f32)
            c2t = sb.tile(P19, dtype=f32)
            s2t = sb.tile(P19, dtype=f32)
            u1 = sb.tile(P19, dtype=f32)
            u2 = sb.tile(P19, dtype=f32)
            u3 = sb.tile(P19, dtype=f32)
            u4 = sb.tile(P19, dtype=f32)
            u5 = sb.tile(P19, dtype=f32)
            w1 = sb.tile(P19, dtype=f32)
            w2 = sb.tile(P19, dtype=f32)
            w3 = sb.tile(P19, dtype=f32)
            w4 = sb.tile(P19, dtype=f32)
            d1 = sb.tile(P19, dtype=f32)
            d2 = sb.tile(P19, dtype=f32)
            rec = sb.tile(P19, dtype=f32)
            tmpv = sb.tile(P19, dtype=f32)
            tmpw = sb.tile(P19, dtype=f32)

            # --- iter 0: evaluate at the grid point (delta = 0) ---
            vstt(c2t[:], cgg[:], 2.0, cgg[:], OP.mult, OP.mult)
            vs(c2t[:], c2t[:], -1.0, None, OP.add)
            vstt(s2t[:], sgg[:], 2.0, cgg[:], OP.mult, OP.mult)
            vt(u1[:], s2t[:], m2Uw[:], OP.mult)
            vt(u2[:], c2t[:], p2Vw[:], OP.mult)
            pt_(u3[:], nP2, sgg[:], OP.mult)
            pt_(u4[:], nQ2, cgg[:], OP.mult)
            vt(d1[:], u1[:], u2[:], OP.add)
            pt_(u5[:], u3[:], u4[:], OP.subtract)
            vt(d1[:], d1[:], u5[:], OP.add)
            vt(w2[:], s2t[:], m4Vw[:], OP.mult)
            pt_(w1[:], c2t[:], m4Uw[:], OP.mult)
            pt_(w3[:], nP2, cgg[:], OP.mult)
            pt_(w4[:], nQ2, sgg[:], OP.mult)
            pt_(w1[:], w1[:], w2[:], OP.add)
            pt_(w3[:], w3[:], w4[:], OP.add)
            pt_(d2[:], w1[:], w3[:], OP.add)
            ps_(d2[:], d2[:], D2_CLAMP, None, OP.max)
            nc.vector.reciprocal(rec[:], d2[:])
            sp0 = sb.tile(P19, dtype=f32)
            vt(sp0[:], d1[:], rec[:], OP.mult)
            vs(sp0[:], sp0[:], -SPACING, SPACING, OP.max, OP.min)
            # --- iter 1: rotate by -sp0 (sin~x, cos~1-x^2/2), reuse rec ---
            vt(x2[:], sp0[:], sp0[:], OP.mult)
            vs(cd[:], x2[:], C2, 1.0, OP.mult, OP.add)
            vt(tmpv[:], sgg[:], sp0[:], OP.mult)
            vt(ct[:], cgg[:], cd[:], OP.mult)
            vt(ct[:], ct[:], tmpv[:], OP.add)
            pt_(st[:], sgg[:], cd[:], OP.mult)
            pt_(tmpw[:], cgg[:], sp0[:], OP.mult)
            pt_(st[:], st[:], tmpw[:], OP.subtract)
            vstt(c2t[:], ct[:], 2.0, ct[:], OP.mult, OP.mult)
            vs(c2t[:], c2t[:], -1.0, None, OP.add)
            vstt(s2t[:], st[:], 2.0, ct[:], OP.mult, OP.mult)
            vt(u1[:], s2t[:], m2Uw[:], OP.mult)
            vt(u2[:], c2t[:], p2Vw[:], OP.mult)
            pt_(u3[:], nP2, st[:], OP.mult)
            pt_(u4[:], nQ2, ct[:], OP.mult)
            vt(d1[:], u1[:], u2[:], OP.add)
            pt_(u5[:], u3[:], u4[:], OP.subtract)
            vt(d1[:], d1[:], u5[:], OP.add)
            sp1 = sb.tile(P19, dtype=f32)
            vt(sp1[:], d1[:], rec[:], OP.mult)
            vs(sp1[:], sp1[:], -SPACING, SPACING, OP.max, OP.min)
            # total delta = -(sp0 + sp1)
            sum01 = sb.tile(P19, dtype=f32)
            vt(sum01[:], sp0[:], sp1[:], OP.add)

            # ---- n* = fold(floor((thg+delta)*N/2pi + 0.5)) ----
            dN = sb.tile(P19, dtype=f32)
            ps_(dN[:], sum01[:], -NFC, 0.5, OP.mult, OP.add)
            nf = sb.tile(P19, dtype=f32)
            vstt(nf[:], tg[:], float(N), dN[:], OP.mult, OP.add)
            ii = sb.tile(P19, dtype=i32)
            nc.vector.tensor_copy(ii[:], nf[:])
            ff = sb.tile(P19, dtype=f32)
            nc.vector.tensor_copy(ff[:], ii[:])
            mgt = sb.tile(P19, dtype=f32)
            vt(mgt[:], ff[:], nf[:], OP.is_gt)
            nst = sb.tile(P19, dtype=f32)
            vt(nst[:], ff[:], mgt[:], OP.subtract)
            mlo = sb.tile(P19, dtype=f32)
            vs(mlo[:], nst[:], 0.0, None, OP.is_lt)
            vstt(nst[:], mlo[:], float(N), nst[:], OP.mult, OP.add)
            nc.gpsimd.dma_start(out=OUT[:], in_=nst[:])
    return nc


_NC_CACHE = None


def _get_nc():
    global _NC_CACHE
    if _NC_CACHE is None:
        _NC_CACHE = _build()
    return _NC_CACHE


def _host_consts():
    import ml_dtypes
    g = np.arange(G, dtype=np.float64)
    tg = 2.0 * np.pi * g / G
    GRIDC = np.stack([np.cos(tg), np.sin(tg)]).astype(ml_dtypes.bfloat16)
    C2Gv = np.cos(2.0 * tg).astype(np.float32)[None, :]
    S2Gv = np.sin(2.0 * tg).astype(np.float32)[None, :]
    return GRIDC, C2Gv, S2Gv


def _make_core_inputs(gt_depths, ModFs, DemodFs):
    GRIDC, C2Gv, S2Gv = _host_consts()
    MODRh = np.ascontiguousarray(ModFs, dtype=np.float32).reshape(125, 240)
    DEMRh = np.ascontiguousarray(DemodFs, dtype=np.float32).reshape(125, 240)
    COMBh = np.ascontiguousarray(
        np.concatenate([MODRh, DEMRh], axis=1))
    flat = np.asarray(gt_depths, dtype=np.float32).reshape(-1)
    per = flat.reshape(NCORES, PPC)
    full = np.concatenate([per, np.zeros((NCORES, NT * 128 - PPC), np.float32)], axis=1)
    gins = full.reshape(NCORES, NT, 128).transpose(0, 2, 1)   # [8,128,19]
    o125 = np.ones((125, 1), np.float32)
    o1 = np.ones((1, 128), np.float32)
    ins = []
    for c in range(NCORES):
        ins.append({
            "GIN": np.ascontiguousarray(gins[c]),
            "COMB": COMBh, "O125": o125, "O1": o1,
            "GRIDC": GRIDC, "C2G": C2Gv, "S2G": S2Gv,
        })
    return ins


def kernel(gt_depths: np.ndarray, ModFs: np.ndarray, DemodFs: np.ndarray) -> np.ndarray:
    nc = _get_nc()
    ins = _make_core_inputs(gt_depths, ModFs, DemodFs)
    res = run_bass_kernel_spmd(nc, ins, core_ids=list(range(NCORES)))
    outs = np.stack([np.asarray(res.results[c]["OUT"]) for c in range(NCORES)])
    out = outs.transpose(0, 2, 1).reshape(NCORES, NT * 128)[:, :PPC].reshape(-1)
    return out.reshape(gt_depths.shape).astype(np.float32)
